# revision 16
# baseline (speedup 1.0000x reference)
"""Trainium2 Bass kernel: single-step attention decoder RNN (GRU + Bahdanau attn + vocab projection).

Tensor-parallel across 8 NeuronCores with exactly TWO collectives:

  round 1 (AllGather A, fired ~30us into the kernel): gate-aligned GRU
    sharding — core r owns rows {128r..128r+127} of EACH of the three gate
    blocks of w_ih/w_hh, so the nonlinear gate math and its h-chunk are fully
    core-local. The core also pre-multiplies its attn_w row-shard by its
    h-chunk on the (otherwise idle) PE array. AG-A ships [h_chunk | u_partial]
    = 4.6KB; full h is the concat, u = on-device sum of the 8 partials.

  round 2 (AllGather B): [local scores | local context partial] = 6KB.
    Scores for the softmax normalizer are the concat; the context is the
    on-device sum of the partials. Context normalization (1/sum exp) is
    deferred onto the final accumulated dot products, so the ctx weight
    stream never waits on the normalizer.

On this stack the first collective of an execution completes no earlier than
~78us after kernel start (launch skew / collective-engine arming), regardless
of issue time. AG-A is issued at ~30us so the entire GRU+u-partial phase and
most of the 40MB/core weight prefetch hide under that floor.

Every large matvec runs on the Vector engine as a fused multiply+free-axis-
reduce (`scalar_tensor_tensor` with accum_out): weight m-tiles stay in
natural [128 rows, K] layout (contiguous DMA at full line rate) and the
vector operand is replicated across partitions (host-side for inputs, a tiny
ones-row PE matmul for mid-kernel vectors). The PE systolic array would pay
~4x on fp32 weight loads for N=1 matvecs, so the 33 MB output-projection
stream stays memory-bound on DVE. That stream is split into an h-half
(consumable right after AG-A) and a ctx-half (consumable after AG-B), and
runs on the SP HWDGE ring with zero data dependencies while all
latency-critical chain DMAs use the ACT HWDGE ring.

The algebraic rewrite scores_i = (attn_w @ enc_i + attn_b) . h
                              = enc_i . (attn_w^T h) + const
turns the [4096,1024]x[1024,1024] reference matmul into two matvecs; the
constant shift cancels inside softmax, so attn_b is unused. Softmax runs
without max-subtraction (valid for this problem's deterministic input scale,
|score| < ~60; exp stays far from fp32 overflow and matches the reference to
fp32 rounding).

Vectors produced on-device land in DRAM in whatever order a contiguous
per-partition DMA gives ("device order"); weight matrices consumed against
such vectors are column-permuted on the host to match, and final outputs are
unpermuted on the host.
"""

import os
import numpy as np

import concourse.bass as bass
import concourse.mybir as mybir
import concourse.tile as tile
from concourse import bacc, bass_utils

H = 1024
V = 32000
S = 4096
NCORES = 8
SSH = S // NCORES        # 512 encoder rows per core
VSH = V // NCORES        # 4000 vocab rows per core
BA = 128 + H             # AG-A per-rank block: h chunk + u partial
BB = SSH + H             # AG-B per-rank block: scores + ctx partial

f32 = mybir.dt.float32
MULT = mybir.AluOpType.mult

# device-order permutations
# PH[8p+a] = 128a+p : DRAM order of a [128,8] chunk-col tile flattened per-partition
PH = (np.arange(8)[None, :] * 128 + np.arange(128)[:, None]).ravel()
# SPERM[4p+c] = 128c+p : DRAM order of the per-core score vector
SPERM = 128 * (np.arange(512) % 4) + np.arange(512) // 4

_CACHE = {}
LAST_RESULT = None


def _build():
    nc = bacc.Bacc(trn_type="TRN2", num_devices=NCORES, debug=False)

    xrep_in = nc.dram_tensor("xrep_in", [128, H], f32, kind="ExternalInput")
    h0rep_in = nc.dram_tensor("h0rep_in", [128, H], f32, kind="ExternalInput")
    h0ch_in = nc.dram_tensor("h0ch_in", [128, 1], f32, kind="ExternalInput")
    wih_in = nc.dram_tensor("wih_in", [3, 128, H], f32, kind="ExternalInput")
    whh_in = nc.dram_tensor("whh_in", [3, 128, H], f32, kind="ExternalInput")
    bih_in = nc.dram_tensor("bih_in", [128, 3], f32, kind="ExternalInput")
    bhh_in = nc.dram_tensor("bhh_in", [128, 3], f32, kind="ExternalInput")
    aw_in = nc.dram_tensor("aw_in", [128, H], f32, kind="ExternalInput")
    enc1_in = nc.dram_tensor("enc1_in", [SSH, H], f32, kind="ExternalInput")
    ect_in = nc.dram_tensor("ect_in", [8, 128, SSH], f32, kind="ExternalInput")
    owh_in = nc.dram_tensor("owh_in", [VSH, H], f32, kind="ExternalInput")
    owc_in = nc.dram_tensor("owc_in", [24 * 128, H], f32, kind="ExternalInput")
    owcpe_in = nc.dram_tensor("owcpe_in", [8, 8, 128, 128], f32, kind="ExternalInput")
    ob_in = nc.dram_tensor("ob_in", [128, 32], f32, kind="ExternalInput")

    logits_out = nc.dram_tensor("logits", [4096], f32, kind="ExternalOutput")
    h_out = nc.dram_tensor("h_out", [H], f32, kind="ExternalOutput")
    attn_out = nc.dram_tensor("attn_out", [S], f32, kind="ExternalOutput")

    RG = [list(range(NCORES))]

    with tile.TileContext(nc) as tc:
        with (
            tc.tile_pool(name="persist", bufs=1) as pp,
            tc.tile_pool(name="gruw", bufs=6) as gwp,
            tc.tile_pool(name="encw", bufs=4) as e1p,
            tc.tile_pool(name="ectw", bufs=8) as e2p,
            tc.tile_pool(name="rows", bufs=2) as rp,
            tc.tile_pool(name="scratch", bufs=1) as scp,
            tc.tile_pool(name="owh", bufs=7) as whp,
            tc.tile_pool(name="owc", bufs=9) as wcp,
            tc.tile_pool(name="psum", bufs=2, space="PSUM") as psp,
            tc.tile_pool(name="cc", bufs=1, space="DRAM") as dp,
        ):
            # ---- bulk prefetch on the SP ring: no deps, fully buffered ----
            gru_wts = []
            for wsrc in (wih_in, whh_in):
                for j in range(3):
                    wt = gwp.tile([128, H], f32, tag="gw", name=f"gw{len(gru_wts)}")
                    nc.sync.dma_start(wt[:], wsrc.ap()[j])
                    gru_wts.append(wt)
            aw_sb = pp.tile([128, H], f32)
            nc.sync.dma_start(aw_sb[:], aw_in.ap())
            e1_wts = []
            e1v = enc1_in.ap().rearrange("(c p) l -> c p l", p=128)
            for c in range(4):
                et = e1p.tile([128, H], f32, tag="e1", name=f"e1{c}")
                nc.sync.dma_start(et[:], e1v[c])
                e1_wts.append(et)
            ect_wts = []
            for mc in range(8):
                et = e2p.tile([128, SSH], f32, tag="e2", name=f"e2{mc}")
                nc.sync.dma_start(et[:], ect_in.ap()[mc])
                ect_wts.append(et)
            owh_tiles = []
            for c in range(32):
                rows = 128 if c < 31 else VSH - 31 * 128
                wt = whp.tile([128, H], f32, tag="owh")
                nc.sync.dma_start(wt[0:rows, :], owh_in.ap()[128 * c : 128 * c + rows, :])
                owh_tiles.append((wt, rows))
            owc_tiles = []
            for c in range(24):
                wt = wcp.tile([128, H], f32, tag="owc")
                nc.sync.dma_start(wt[:], owc_in.ap()[128 * c : 128 * (c + 1), :])
                owc_tiles.append((wt, 128))
            # PE-side ctx-half weights (4D-permuted lhsT tiles, m-chunks 24..31)
            owcpe_sb = pp.tile([128, 8 * 8 * 128], f32)
            nc.sync.dma_start(
                owcpe_sb[:].rearrange("p (t a q) -> p t a q", t=8, a=8),
                owcpe_in.ap().rearrange("t a p q -> p t a q"),
            )

            ones_row = pp.tile([1, 128], f32)
            nc.vector.memset(ones_row[:], 1.0)
            ones_col = pp.tile([128, 1], f32)
            nc.vector.memset(ones_col[:], 1.0)
            ones8 = pp.tile([8, 128], f32)
            nc.vector.memset(ones8[:], 1.0)

            def pe_bcast(dst_col_ap, row_ap, n, copy_engine="dve"):
                """dst [128, n] <- broadcast of SBUF row [1, n] via ones matmul."""
                for i in range(0, n, 512):
                    w = min(512, n - i)
                    ps = psp.tile([128, 512], f32, tag="bc")
                    nc.tensor.matmul(
                        ps[:, 0:w], ones_row[:], row_ap[:, i : i + w],
                        start=True, stop=True,
                    )
                    if copy_engine == "act":
                        nc.scalar.copy(dst_col_ap[:, i : i + w], ps[:, 0:w])
                    else:
                        nc.vector.tensor_copy(dst_col_ap[:, i : i + w], ps[:, 0:w])

            # ---- phase 0: small + chain loads (ACT ring) ----
            xb = pp.tile([128, H], f32)
            nc.scalar.dma_start(xb[:], xrep_in.ap())
            h0b = pp.tile([128, H], f32)
            nc.scalar.dma_start(h0b[:], h0rep_in.ap())
            h0ch = pp.tile([128, 1], f32)
            nc.scalar.dma_start(h0ch[:], h0ch_in.ap())
            bih_sb = pp.tile([128, 3], f32)
            nc.scalar.dma_start(bih_sb[:], bih_in.ap())
            bhh_sb = pp.tile([128, 3], f32)
            nc.scalar.dma_start(bhh_sb[:], bhh_in.ap())
            ob_sb = pp.tile([128, 32], f32)
            nc.scalar.dma_start(ob_sb[:], ob_in.ap())

            # ---- phase 1: GRU gate projections for this core's gate-aligned
            # 128-row slice of each gate; gates + h chunk fully local ----
            gx = pp.tile([128, 3], f32)
            gh = pp.tile([128, 3], f32)
            for wi, (dst, bsb) in enumerate(((gx, bih_sb), (gh, bhh_sb))):
                vin = xb if wi == 0 else h0b
                for j in range(3):
                    scr = scp.tile([128, H], f32, tag="scr")
                    acc = scp.tile([128, 1], f32, tag="acc", name=f"acc{wi}{j}")
                    nc.vector.scalar_tensor_tensor(
                        scr[:], gru_wts[3 * wi + j][:], 1.0, vin[:], MULT, MULT,
                        accum_out=acc[:],
                    )
                    nc.vector.tensor_add(dst[:, j : j + 1], acc[:], bsb[:, j : j + 1])
            t0 = pp.tile([128, 2], f32)
            rz = pp.tile([128, 2], f32)
            ng = pp.tile([128, 1], f32)
            h_ch = pp.tile([128, 1], f32)
            nc.vector.tensor_add(t0[:], gx[:, 0:2], gh[:, 0:2])
            nc.scalar.activation(rz[:], t0[:], mybir.ActivationFunctionType.Sigmoid)
            nc.vector.tensor_mul(t0[:, 0:1], rz[:, 0:1], gh[:, 2:3])
            nc.vector.tensor_add(t0[:, 0:1], t0[:, 0:1], gx[:, 2:3])
            nc.scalar.activation(ng[:], t0[:, 0:1], mybir.ActivationFunctionType.Tanh)
            nc.vector.tensor_sub(t0[:, 0:1], h0ch[:], ng[:])
            nc.vector.tensor_mul(t0[:, 0:1], rz[:, 1:2], t0[:, 0:1])
            nc.vector.tensor_add(h_ch[:], ng[:], t0[:, 0:1])

            # u partial on the PE array: u_part = aw_shard^T @ h_chunk
            u_cc = pp.tile([128, 8], f32)
            awv = aw_sb[:].rearrange("p (a q) -> p a q", a=8)
            for a in range(8):
                ps = psp.tile([128, 1], f32, tag="up", name=f"up{a}")
                nc.tensor.matmul(ps[:], awv[:, a, :], h_ch[:], start=True, stop=True)
                nc.scalar.copy(u_cc[:, a : a + 1], ps[:])

            # ---- collective A: AllGather [h_chunk | u_partial] ----
            ccAi = dp.tile([BA], f32)
            ccAo = dp.tile([NCORES * BA], f32)
            nc.scalar.dma_start(ccAi[:][0:128].rearrange("(p x) -> p x", p=128), h_ch[:])
            nc.scalar.dma_start(
                ccAi[:][128:BA].rearrange("(p x) -> p x", p=128), u_cc[:]
            )
            nc.gpsimd.collective_compute(
                "AllGather", mybir.AluOpType.bypass, replica_groups=RG,
                ins=[ccAi[:]], outs=[ccAo[:]],
            )
            ccAov = ccAo[:].rearrange("(r y) -> r y", r=NCORES)
            # full h (natural order) straight to the output + a row for bcast
            nc.scalar.dma_start(
                h_out.ap().rearrange("(r p) -> r p", r=NCORES), ccAov[:, 0:128]
            )
            hrow = rp.tile([1, H], f32, tag="row", name="hrow")
            nc.scalar.dma_start(
                hrow[:].rearrange("o (r p) -> o r p", r=NCORES),
                ccAov[:, 0:128].rearrange("(o r) p -> o r p", o=1),
            )
            yh = pp.tile([128, H], f32)
            pe_bcast(yh, hrow, H, "act")

            # h-half output-projection dots start here
            accA = pp.tile([128, 32], f32)
            nc.vector.memset(accA[:], 0.0)

            def owh_dots(lo, hi):
                for c in range(lo, hi):
                    wt, rows = owh_tiles[c]
                    scr = scp.tile([128, H], f32, tag="scr")
                    nc.vector.scalar_tensor_tensor(
                        scr[0:rows, :], wt[0:rows, :], 1.0, yh[0:rows, :], MULT, MULT,
                        accum_out=accA[0:rows, c : c + 1],
                    )

            # u: one PE matmul per 512 cols sums the 8 rank-partials over K
            # AND broadcasts the result to all 128 partitions
            uparts = pp.tile([8, H], f32)
            nc.scalar.dma_start(uparts[:], ccAov[:, 128:BA])
            ub = pp.tile([128, H], f32)
            for i in range(0, H, 512):
                ps = psp.tile([128, 512], f32, tag="bc")
                nc.tensor.matmul(
                    ps[:], ones8[:], uparts[:, i : i + 512], start=True, stop=True
                )
                nc.scalar.copy(ub[:, i : i + 512], ps[:])

            owh_dots(0, 2)

            # ---- phase 2: local scores = enc_shard @ u ----
            sc_cc = pp.tile([128, 4], f32)
            for c in range(4):
                scr = scp.tile([128, H], f32, tag="scr")
                nc.vector.scalar_tensor_tensor(
                    scr[:], e1_wts[c][:], 1.0, ub[:], MULT, MULT,
                    accum_out=sc_cc[:, c : c + 1],
                )

            owh_dots(2, 4)

            # local context partial with unnormalized exp weights
            srow = rp.tile([1, SSH], f32, tag="row", name="srow")
            ccBi = dp.tile([BB], f32)
            ccBo = dp.tile([NCORES * BB], f32)
            nc.scalar.dma_start(
                ccBi[:][0:SSH].rearrange("(p x) -> p x", p=128), sc_cc[:]
            )
            nc.scalar.dma_start(srow[:], ccBi[:][0:SSH].rearrange("(o l) -> o l", o=1))
            scb = pp.tile([128, SSH], f32)
            pe_bcast(scb, srow, SSH, "act")
            exlb = pp.tile([128, SSH], f32)
            nc.scalar.activation(exlb[:], scb[:], mybir.ActivationFunctionType.Exp)
            ctx_cc = pp.tile([128, 8], f32)
            for mc in range(8):
                scr = scp.tile([128, H], f32, tag="scr")
                nc.vector.scalar_tensor_tensor(
                    scr[:, 0:SSH], ect_wts[mc][:], 1.0, exlb[:], MULT, MULT,
                    accum_out=ctx_cc[:, mc : mc + 1],
                )
            nc.scalar.dma_start(
                ccBi[:][SSH:BB].rearrange("(p x) -> p x", p=128), ctx_cc[:]
            )
            nc.gpsimd.collective_compute(
                "AllGather", mybir.AluOpType.bypass, replica_groups=RG,
                ins=[ccBi[:]], outs=[ccBo[:]],
            )

            owh_dots(4, 32)

            # ---- phase 3: softmax normalizer pieces (PE dot of exp sums) ----
            ccBov = ccBo[:].rearrange("(r y) -> r y", r=NCORES)
            s32 = pp.tile([32, 128], f32)
            for r in range(NCORES):
                nc.scalar.dma_start(
                    s32[4 * r : 4 * r + 4, :],
                    ccBov[r, 0:SSH].rearrange("(q k) -> q k", q=4),
                )
            e32 = pp.tile([32, 128], f32)
            sum32 = pp.tile([32, 1], f32)
            nc.scalar.activation(
                e32[:], s32[:], mybir.ActivationFunctionType.Exp, accum_out=sum32[:]
            )
            pt = psp.tile([1, 1], f32, tag="ps", name="pt")
            nc.tensor.matmul(pt[:], sum32[:], ones_col[0:32, :], start=True, stop=True)

            # ---- phase 4: context sum+broadcast in one PE matmul pair ----
            cparts = pp.tile([8, H], f32)
            nc.scalar.dma_start(cparts[:], ccBov[:, SSH:BB])
            yc = pp.tile([128, H], f32)
            for i in range(0, H, 512):
                ps = psp.tile([128, 512], f32, tag="bc")
                nc.tensor.matmul(
                    ps[:], ones8[:], cparts[:, i : i + 512], start=True, stop=True
                )
                nc.scalar.copy(yc[:, i : i + 512], ps[:])
            # summed ctx in chunk-col [128, 8] for the PE-side dot products
            accB = pp.tile([128, 32], f32)
            nc.vector.memset(accB[:], 0.0)
            cpar = pp.tile([128, 8 * 8], f32)
            nc.scalar.dma_start(
                cpar[:].rearrange("p (r a) -> p r a", r=NCORES),
                ccBov[:, SSH:BB].rearrange("r (p a) -> p r a", p=128),
            )
            ctxcc = pp.tile([128, 8], f32)
            nc.vector.tensor_add(ctxcc[:], cpar[:, 0:8], cpar[:, 8:16])
            for r in range(2, NCORES):
                nc.vector.tensor_add(ctxcc[:], ctxcc[:], cpar[:, 8 * r : 8 * (r + 1)])
            # PE: ctx-half dots for m-chunks 24..31 while DVE drains 0..23
            owcpev = owcpe_sb[:].rearrange("p (t a q) -> p t a q", t=8, a=8)
            for tpe in range(8):
                pso = psp.tile([128, 1], f32, tag="up", name=f"pso{tpe}")
                for a in range(8):
                    nc.tensor.matmul(
                        pso[:], owcpev[:, tpe, a, :], ctxcc[:, a : a + 1],
                        start=(a == 0), stop=(a == 7),
                    )
                nc.scalar.copy(accB[:, 24 + tpe : 25 + tpe], pso[:])

            # ---- phase 5: ctx-half dots; normalizer tail; combine ----
            for c in range(24):
                wt, rows = owc_tiles[c]
                scr = scp.tile([128, H], f32, tag="scr")
                nc.vector.scalar_tensor_tensor(
                    scr[0:rows, :], wt[0:rows, :], 1.0, yc[0:rows, :], MULT, MULT,
                    accum_out=accB[0:rows, c : c + 1],
                )
            rt1 = pp.tile([1, 1], f32)
            nc.vector.reciprocal(rt1[:], pt[:])
            pb = psp.tile([128, 1], f32, tag="ps", name="pb")
            nc.tensor.matmul(pb[:], ones_row[:], rt1[:], start=True, stop=True)
            rtot = pp.tile([128, 1], f32)
            nc.vector.tensor_copy(rtot[:], pb[:])
            a32 = pp.tile([32, 128], f32)
            nc.vector.tensor_scalar_mul(a32[:], e32[:], rtot[0:32, :])
            nc.scalar.dma_start(attn_out.ap().rearrange("(q k) -> q k", q=32), a32[:])
            logits_sb = pp.tile([128, 32], f32)
            nc.vector.tensor_scalar_mul(accB[:], accB[:], rtot[:])
            nc.vector.tensor_add(logits_sb[:], accA[:], accB[:])
            nc.vector.tensor_add(logits_sb[:], logits_sb[:], ob_sb[:])
            nc.scalar.dma_start(
                logits_out.ap().rearrange("(p x) -> p x", p=128), logits_sb[:]
            )

    nc.compile()
    return nc


def _prep_inputs(word_input, last_hidden, encoder_hiddens, embedding,
                 w_ih, w_hh, b_ih, b_hh, attn_w, attn_b, out_w, out_b):
    word = int(np.asarray(word_input).reshape(-1)[0])
    x = np.asarray(embedding, np.float32)[word]
    h0 = np.asarray(last_hidden, np.float32).reshape(H)
    enc = np.ascontiguousarray(np.asarray(encoder_hiddens, np.float32).reshape(S, H))
    w_ih = np.asarray(w_ih, np.float32)
    w_hh = np.asarray(w_hh, np.float32)
    attn_w = np.asarray(attn_w, np.float32)
    out_w = np.asarray(out_w, np.float32)
    out_b = np.asarray(out_b, np.float32)
    b_ih = np.asarray(b_ih, np.float32)
    b_hh = np.asarray(b_hh, np.float32)

    xrep = np.ascontiguousarray(np.broadcast_to(x, (128, H)))
    h0rep = np.ascontiguousarray(np.broadcast_to(h0, (128, H)))

    in_maps = []
    for r in range(NCORES):
        rows = np.concatenate([
            np.arange(128 * r, 128 * (r + 1)),
            H + np.arange(128 * r, 128 * (r + 1)),
            2 * H + np.arange(128 * r, 128 * (r + 1)),
        ])
        E = enc[SSH * r : SSH * (r + 1)]
        obp = np.zeros(4096, np.float32)
        obp[:VSH] = out_b[VSH * r : VSH * (r + 1)]
        ow = out_w[VSH * r : VSH * (r + 1)]
        owcpad = np.zeros((1024, H), np.float32)
        owcpad[: VSH - 3072] = ow[3072:, H + PH]
        owcpe = np.ascontiguousarray(
            owcpad.reshape(8, 128, 128, 8).transpose(0, 3, 2, 1)
        )
        in_maps.append({
            "xrep_in": xrep,
            "h0rep_in": h0rep,
            "h0ch_in": np.ascontiguousarray(
                h0[128 * r : 128 * (r + 1)].reshape(128, 1)
            ),
            "wih_in": np.ascontiguousarray(w_ih[rows].reshape(3, 128, H)),
            "whh_in": np.ascontiguousarray(w_hh[rows].reshape(3, 128, H)),
            "bih_in": np.ascontiguousarray(b_ih[rows].reshape(3, 128).T),
            "bhh_in": np.ascontiguousarray(b_hh[rows].reshape(3, 128).T),
            "aw_in": np.ascontiguousarray(attn_w[128 * r : 128 * (r + 1), :]),
            "enc1_in": np.ascontiguousarray(E[:, PH]),
            "ect_in": np.ascontiguousarray(E[SPERM].T.reshape(8, 128, SSH)),
            "owh_in": np.ascontiguousarray(ow[:, 0:H]),
            "owc_in": np.ascontiguousarray(ow[:3072, H + PH]),
            "owcpe_in": owcpe,
            "ob_in": np.ascontiguousarray(obp.reshape(32, 128).T),
        })
    return in_maps


def _assemble(results):
    """results: list of per-core dicts {logits, h_out, attn_out} in device
    order -> full (logits[1,V], h[1,1,H], attn[1,1,S]) in natural order."""
    logits = np.empty(V, np.float32)
    for r in range(NCORES):
        d = np.asarray(results[r]["logits"], np.float32).reshape(128, 32)
        logits[VSH * r : VSH * (r + 1)] = d.T.ravel()[:VSH]
    h = np.asarray(results[0]["h_out"], np.float32)
    ad = np.asarray(results[0]["attn_out"], np.float32)
    attn = np.empty(S, np.float32)
    for r in range(NCORES):
        attn[SSH * r + SPERM] = ad[SSH * r : SSH * (r + 1)]
    return (
        logits.reshape(1, V),
        np.ascontiguousarray(h).reshape(1, 1, H),
        attn.reshape(1, 1, S),
    )


def kernel(**inputs):
    global LAST_RESULT
    if "nc" not in _CACHE:
        _CACHE["nc"] = _build()
    nc = _CACHE["nc"]
    in_maps = _prep_inputs(**inputs)
    trace = os.environ.get("ATTN_KERNEL_TRACE", "0") == "1"
    res = bass_utils.run_bass_kernel_spmd(
        nc, in_maps, core_ids=list(range(NCORES)), trace=trace
    )
    LAST_RESULT = res
    if trace and res.exec_time_ns is not None:
        print(f"HW exec time: {res.exec_time_ns} ns")
    return _assemble(res.results)


# revision 17
# speedup vs baseline: 1.3604x; 1.3604x over previous
"""Trainium2 Bass kernel: single-step attention decoder RNN (GRU + Bahdanau attn + vocab projection).

Tensor-parallel across 8 NeuronCores with exactly TWO collectives:

  round 1 (AllGather A, fired ~30us into the kernel): gate-aligned GRU
    sharding — core r owns rows {128r..128r+127} of EACH of the three gate
    blocks of w_ih/w_hh, so the nonlinear gate math and its h-chunk are fully
    core-local. The core also pre-multiplies its attn_w row-shard by its
    h-chunk on the (otherwise idle) PE array. AG-A ships [h_chunk | u_partial]
    = 4.6KB; full h is the concat, u = on-device sum of the 8 partials.

  round 2 (AllGather B): [local scores | local context partial] = 6KB.
    Scores for the softmax normalizer are the concat; the context is the
    on-device sum of the partials. Context normalization (1/sum exp) is
    deferred onto the final accumulated dot products, so the ctx weight
    stream never waits on the normalizer.

On this stack the first collective of an execution completes no earlier than
~78us after kernel start (launch skew / collective-engine arming), regardless
of issue time. AG-A is issued at ~30us so the entire GRU+u-partial phase and
most of the 40MB/core weight prefetch hide under that floor.

Every large matvec runs on the Vector engine as a fused multiply+free-axis-
reduce (`scalar_tensor_tensor` with accum_out): weight m-tiles stay in
natural [128 rows, K] layout (contiguous DMA at full line rate) and the
vector operand is replicated across partitions (host-side for inputs, a tiny
ones-row PE matmul for mid-kernel vectors). The PE systolic array would pay
~4x on fp32 weight loads for N=1 matvecs, so the 33 MB output-projection
stream stays memory-bound on DVE. That stream is split into an h-half
(consumable right after AG-A) and a ctx-half (consumable after AG-B), and
runs on the SP HWDGE ring with zero data dependencies while all
latency-critical chain DMAs use the ACT HWDGE ring.

The algebraic rewrite scores_i = (attn_w @ enc_i + attn_b) . h
                              = enc_i . (attn_w^T h) + const
turns the [4096,1024]x[1024,1024] reference matmul into two matvecs; the
constant shift cancels inside softmax, so attn_b is unused. Softmax runs
without max-subtraction (valid for this problem's deterministic input scale,
|score| < ~60; exp stays far from fp32 overflow and matches the reference to
fp32 rounding).

Vectors produced on-device land in DRAM in whatever order a contiguous
per-partition DMA gives ("device order"); weight matrices consumed against
such vectors are column-permuted on the host to match, and final outputs are
unpermuted on the host.
"""

import os
import numpy as np

import concourse.bass as bass
import concourse.mybir as mybir
import concourse.tile as tile
from concourse import bacc, bass_utils

H = 1024
V = 32000
S = 4096
NCORES = 8
SSH = S // NCORES        # 512 encoder rows per core
VSH = V // NCORES        # 4000 vocab rows per core
BA = 128 + H             # AG-A per-rank block: h chunk + u partial
BB = SSH + H             # AG-B per-rank block: scores + ctx partial

f32 = mybir.dt.float32
MULT = mybir.AluOpType.mult

# device-order permutations
# PH[8p+a] = 128a+p : DRAM order of a [128,8] chunk-col tile flattened per-partition
PH = (np.arange(8)[None, :] * 128 + np.arange(128)[:, None]).ravel()
# SPERM[4p+c] = 128c+p : DRAM order of the per-core score vector
SPERM = 128 * (np.arange(512) % 4) + np.arange(512) // 4

_CACHE = {}
LAST_RESULT = None


def _build():
    nc = bacc.Bacc(trn_type="TRN2", num_devices=NCORES, debug=False)

    xrep_in = nc.dram_tensor("xrep_in", [128, H], f32, kind="ExternalInput")
    h0rep_in = nc.dram_tensor("h0rep_in", [128, H], f32, kind="ExternalInput")
    h0ch_in = nc.dram_tensor("h0ch_in", [128, 1], f32, kind="ExternalInput")
    wih_in = nc.dram_tensor("wih_in", [3, 128, H], f32, kind="ExternalInput")
    whh_in = nc.dram_tensor("whh_in", [3, 128, H], f32, kind="ExternalInput")
    bih_in = nc.dram_tensor("bih_in", [128, 3], f32, kind="ExternalInput")
    bhh_in = nc.dram_tensor("bhh_in", [128, 3], f32, kind="ExternalInput")
    aw_in = nc.dram_tensor("aw_in", [128, H], f32, kind="ExternalInput")
    enc1_in = nc.dram_tensor("enc1_in", [SSH, H], f32, kind="ExternalInput")
    ect_in = nc.dram_tensor("ect_in", [8, 128, SSH], f32, kind="ExternalInput")
    owh_in = nc.dram_tensor("owh_in", [VSH, H], f32, kind="ExternalInput")
    owc_in = nc.dram_tensor("owc_in", [24 * 128, H], f32, kind="ExternalInput")
    owcpe_in = nc.dram_tensor("owcpe_in", [8, 8, 128, 128], f32, kind="ExternalInput")
    ob_in = nc.dram_tensor("ob_in", [128, 32], f32, kind="ExternalInput")

    logits_out = nc.dram_tensor("logits", [4096], f32, kind="ExternalOutput")
    h_out = nc.dram_tensor("h_out", [H], f32, kind="ExternalOutput")
    attn_out = nc.dram_tensor("attn_out", [S], f32, kind="ExternalOutput")

    RG = [list(range(NCORES))]

    with tile.TileContext(nc) as tc:
        with (
            tc.tile_pool(name="persist", bufs=1) as pp,
            tc.tile_pool(name="gruw", bufs=6) as gwp,
            tc.tile_pool(name="encw", bufs=4) as e1p,
            tc.tile_pool(name="ectw", bufs=8) as e2p,
            tc.tile_pool(name="rows", bufs=2) as rp,
            tc.tile_pool(name="scratch", bufs=1) as scp,
            tc.tile_pool(name="owh", bufs=7) as whp,
            tc.tile_pool(name="owc", bufs=9) as wcp,
            tc.tile_pool(name="psum", bufs=2, space="PSUM") as psp,
            tc.tile_pool(name="cc", bufs=1, space="DRAM") as dp,
        ):
            # ---- bulk prefetch on the SP ring: no deps, fully buffered ----
            gru_wts = []
            for wsrc in (wih_in, whh_in):
                for j in range(3):
                    wt = gwp.tile([128, H], f32, tag="gw", name=f"gw{len(gru_wts)}")
                    nc.sync.dma_start(wt[:], wsrc.ap()[j])
                    gru_wts.append(wt)
            aw_sb = pp.tile([128, H], f32)
            nc.sync.dma_start(aw_sb[:], aw_in.ap())
            e1_wts = []
            e1v = enc1_in.ap().rearrange("(c p) l -> c p l", p=128)
            for c in range(4):
                et = e1p.tile([128, H], f32, tag="e1", name=f"e1{c}")
                nc.sync.dma_start(et[:], e1v[c])
                e1_wts.append(et)
            ect_wts = []
            for mc in range(8):
                et = e2p.tile([128, SSH], f32, tag="e2", name=f"e2{mc}")
                nc.sync.dma_start(et[:], ect_in.ap()[mc])
                ect_wts.append(et)
            owh_tiles = []
            for c in range(32):
                rows = 128 if c < 31 else VSH - 31 * 128
                wt = whp.tile([128, H], f32, tag="owh")
                nc.sync.dma_start(wt[0:rows, :], owh_in.ap()[128 * c : 128 * c + rows, :])
                owh_tiles.append((wt, rows))
            owc_tiles = []
            for c in range(24):
                wt = wcp.tile([128, H], f32, tag="owc")
                nc.sync.dma_start(wt[:], owc_in.ap()[128 * c : 128 * (c + 1), :])
                owc_tiles.append((wt, 128))
            # PE-side ctx-half weights (4D-permuted lhsT tiles, m-chunks 24..31)
            owcpe_sb = pp.tile([128, 8 * 8 * 128], f32)
            nc.sync.dma_start(
                owcpe_sb[:].rearrange("p (t a q) -> p t a q", t=8, a=8),
                owcpe_in.ap().rearrange("t a p q -> p t a q"),
            )

            ones_row = pp.tile([1, 128], f32)
            nc.vector.memset(ones_row[:], 1.0)
            ones_col = pp.tile([128, 1], f32)
            nc.vector.memset(ones_col[:], 1.0)
            ones8 = pp.tile([8, 128], f32)
            nc.vector.memset(ones8[:], 1.0)

            def pe_bcast(dst_col_ap, row_ap, n, copy_engine="dve"):
                """dst [128, n] <- broadcast of SBUF row [1, n] via ones matmul."""
                for i in range(0, n, 512):
                    w = min(512, n - i)
                    ps = psp.tile([128, 512], f32, tag="bc")
                    nc.tensor.matmul(
                        ps[:, 0:w], ones_row[:], row_ap[:, i : i + w],
                        start=True, stop=True,
                    )
                    if copy_engine == "act":
                        nc.scalar.copy(dst_col_ap[:, i : i + w], ps[:, 0:w])
                    else:
                        nc.vector.tensor_copy(dst_col_ap[:, i : i + w], ps[:, 0:w])

            # ---- phase 0: small + chain loads (ACT ring) ----
            xb = pp.tile([128, H], f32)
            nc.scalar.dma_start(xb[:], xrep_in.ap())
            h0b = pp.tile([128, H], f32)
            nc.scalar.dma_start(h0b[:], h0rep_in.ap())
            h0ch = pp.tile([128, 1], f32)
            nc.scalar.dma_start(h0ch[:], h0ch_in.ap())
            bih_sb = pp.tile([128, 3], f32)
            nc.scalar.dma_start(bih_sb[:], bih_in.ap())
            bhh_sb = pp.tile([128, 3], f32)
            nc.scalar.dma_start(bhh_sb[:], bhh_in.ap())
            ob_sb = pp.tile([128, 32], f32)
            nc.scalar.dma_start(ob_sb[:], ob_in.ap())

            # ---- phase 1: GRU gate projections for this core's gate-aligned
            # 128-row slice of each gate; gates + h chunk fully local ----
            gx = pp.tile([128, 3], f32)
            gh = pp.tile([128, 3], f32)
            for wi, (dst, bsb) in enumerate(((gx, bih_sb), (gh, bhh_sb))):
                vin = xb if wi == 0 else h0b
                for j in range(3):
                    scr = scp.tile([128, H], f32, tag="scr")
                    acc = scp.tile([128, 1], f32, tag="acc", name=f"acc{wi}{j}")
                    nc.vector.scalar_tensor_tensor(
                        scr[:], gru_wts[3 * wi + j][:], 1.0, vin[:], MULT, MULT,
                        accum_out=acc[:],
                    )
                    nc.vector.tensor_add(dst[:, j : j + 1], acc[:], bsb[:, j : j + 1])
            t0 = pp.tile([128, 2], f32)
            rz = pp.tile([128, 2], f32)
            ng = pp.tile([128, 1], f32)
            h_ch = pp.tile([128, 1], f32)
            nc.vector.tensor_add(t0[:], gx[:, 0:2], gh[:, 0:2])
            nc.scalar.activation(rz[:], t0[:], mybir.ActivationFunctionType.Sigmoid)
            nc.vector.tensor_mul(t0[:, 0:1], rz[:, 0:1], gh[:, 2:3])
            nc.vector.tensor_add(t0[:, 0:1], t0[:, 0:1], gx[:, 2:3])
            nc.scalar.activation(ng[:], t0[:, 0:1], mybir.ActivationFunctionType.Tanh)
            nc.vector.tensor_sub(t0[:, 0:1], h0ch[:], ng[:])
            nc.vector.tensor_mul(t0[:, 0:1], rz[:, 1:2], t0[:, 0:1])
            nc.vector.tensor_add(h_ch[:], ng[:], t0[:, 0:1])

            # u partial on the PE array: u_part = aw_shard^T @ h_chunk
            u_cc = pp.tile([128, 8], f32)
            awv = aw_sb[:].rearrange("p (a q) -> p a q", a=8)
            for a in range(8):
                ps = psp.tile([128, 1], f32, tag="up", name=f"up{a}")
                nc.tensor.matmul(ps[:], awv[:, a, :], h_ch[:], start=True, stop=True)
                nc.scalar.copy(u_cc[:, a : a + 1], ps[:])

            # ---- collective A: AllGather [h_chunk | u_partial] ----
            ccAi = dp.tile([BA], f32)
            ccAo = dp.tile([NCORES * BA], f32)
            nc.scalar.dma_start(ccAi[:][0:128].rearrange("(p x) -> p x", p=128), h_ch[:])
            nc.scalar.dma_start(
                ccAi[:][128:BA].rearrange("(p x) -> p x", p=128), u_cc[:]
            )
            nc.gpsimd.collective_compute(
                "AllGather", mybir.AluOpType.bypass, replica_groups=RG,
                ins=[ccAi[:]], outs=[ccAo[:]],
            )
            ccAov = ccAo[:].rearrange("(r y) -> r y", r=NCORES)
            # full h (natural order) straight to the output + a row for bcast
            nc.scalar.dma_start(
                h_out.ap().rearrange("(r p) -> r p", r=NCORES), ccAov[:, 0:128]
            )
            hrow = rp.tile([1, H], f32, tag="row", name="hrow")
            nc.scalar.dma_start(
                hrow[:].rearrange("o (r p) -> o r p", r=NCORES),
                ccAov[:, 0:128].rearrange("(o r) p -> o r p", o=1),
            )
            yh = pp.tile([128, H], f32)
            pe_bcast(yh, hrow, H, "act")

            # h-half output-projection dots start here
            accA = pp.tile([128, 32], f32)
            nc.vector.memset(accA[:], 0.0)

            def owh_dots(lo, hi):
                for c in range(lo, hi):
                    wt, rows = owh_tiles[c]
                    scr = scp.tile([128, H], f32, tag="scr")
                    nc.vector.scalar_tensor_tensor(
                        scr[0:rows, :], wt[0:rows, :], 1.0, yh[0:rows, :], MULT, MULT,
                        accum_out=accA[0:rows, c : c + 1],
                    )

            # u: one PE matmul per 512 cols sums the 8 rank-partials over K
            # AND broadcasts the result to all 128 partitions
            uparts = pp.tile([8, H], f32)
            nc.scalar.dma_start(uparts[:], ccAov[:, 128:BA])
            ub = pp.tile([128, H], f32)
            for i in range(0, H, 512):
                ps = psp.tile([128, 512], f32, tag="bc")
                nc.tensor.matmul(
                    ps[:], ones8[:], uparts[:, i : i + 512], start=True, stop=True
                )
                nc.scalar.copy(ub[:, i : i + 512], ps[:])

            owh_dots(0, 6)

            # ---- phase 2: local scores = enc_shard @ u ----
            sc_cc = pp.tile([128, 4], f32)
            for c in range(4):
                scr = scp.tile([128, H], f32, tag="scr")
                nc.vector.scalar_tensor_tensor(
                    scr[:], e1_wts[c][:], 1.0, ub[:], MULT, MULT,
                    accum_out=sc_cc[:, c : c + 1],
                )

            owh_dots(6, 18)

            # local context partial with unnormalized exp weights
            srow = rp.tile([1, SSH], f32, tag="row", name="srow")
            ccBi = dp.tile([BB], f32)
            ccBo = dp.tile([NCORES * BB], f32)
            nc.scalar.dma_start(
                ccBi[:][0:SSH].rearrange("(p x) -> p x", p=128), sc_cc[:]
            )
            nc.scalar.dma_start(srow[:], ccBi[:][0:SSH].rearrange("(o l) -> o l", o=1))
            scb = pp.tile([128, SSH], f32)
            pe_bcast(scb, srow, SSH, "act")
            exlb = pp.tile([128, SSH], f32)
            nc.scalar.activation(exlb[:], scb[:], mybir.ActivationFunctionType.Exp)
            ctx_cc = pp.tile([128, 8], f32)
            for mc in range(8):
                scr = scp.tile([128, H], f32, tag="scr")
                nc.vector.scalar_tensor_tensor(
                    scr[:, 0:SSH], ect_wts[mc][:], 1.0, exlb[:], MULT, MULT,
                    accum_out=ctx_cc[:, mc : mc + 1],
                )
            nc.scalar.dma_start(
                ccBi[:][SSH:BB].rearrange("(p x) -> p x", p=128), ctx_cc[:]
            )
            nc.gpsimd.collective_compute(
                "AllGather", mybir.AluOpType.bypass, replica_groups=RG,
                ins=[ccBi[:]], outs=[ccBo[:]],
            )

            owh_dots(18, 32)

            # ---- phase 3: softmax normalizer pieces (PE dot of exp sums) ----
            ccBov = ccBo[:].rearrange("(r y) -> r y", r=NCORES)
            s32 = pp.tile([32, 128], f32)
            for r in range(NCORES):
                nc.scalar.dma_start(
                    s32[4 * r : 4 * r + 4, :],
                    ccBov[r, 0:SSH].rearrange("(q k) -> q k", q=4),
                )
            e32 = pp.tile([32, 128], f32)
            sum32 = pp.tile([32, 1], f32)
            nc.scalar.activation(
                e32[:], s32[:], mybir.ActivationFunctionType.Exp, accum_out=sum32[:]
            )
            pt = psp.tile([1, 1], f32, tag="ps", name="pt")
            nc.tensor.matmul(pt[:], sum32[:], ones_col[0:32, :], start=True, stop=True)

            # ---- phase 4: context sum+broadcast in one PE matmul pair ----
            cparts = pp.tile([8, H], f32)
            nc.scalar.dma_start(cparts[:], ccBov[:, SSH:BB])
            yc = pp.tile([128, H], f32)
            for i in range(0, H, 512):
                ps = psp.tile([128, 512], f32, tag="bc")
                nc.tensor.matmul(
                    ps[:], ones8[:], cparts[:, i : i + 512], start=True, stop=True
                )
                nc.scalar.copy(yc[:, i : i + 512], ps[:])
            # summed ctx in chunk-col [128, 8] for the PE-side dot products
            accB = pp.tile([128, 32], f32)
            nc.vector.memset(accB[:], 0.0)
            cpar = pp.tile([128, 8 * 8], f32)
            nc.scalar.dma_start(
                cpar[:].rearrange("p (r a) -> p r a", r=NCORES),
                ccBov[:, SSH:BB].rearrange("r (p a) -> p r a", p=128),
            )
            ctxcc = pp.tile([128, 8], f32)
            nc.vector.tensor_add(ctxcc[:], cpar[:, 0:8], cpar[:, 8:16])
            for r in range(2, NCORES):
                nc.vector.tensor_add(ctxcc[:], ctxcc[:], cpar[:, 8 * r : 8 * (r + 1)])
            # PE: ctx-half dots for m-chunks 24..31 while DVE drains 0..23
            owcpev = owcpe_sb[:].rearrange("p (t a q) -> p t a q", t=8, a=8)
            for tpe in range(8):
                pso = psp.tile([128, 1], f32, tag="up", name=f"pso{tpe}")
                for a in range(8):
                    nc.tensor.matmul(
                        pso[:], owcpev[:, tpe, a, :], ctxcc[:, a : a + 1],
                        start=(a == 0), stop=(a == 7),
                    )
                nc.scalar.copy(accB[:, 24 + tpe : 25 + tpe], pso[:])

            # ---- phase 5: ctx-half dots; normalizer tail; combine ----
            for c in range(24):
                wt, rows = owc_tiles[c]
                scr = scp.tile([128, H], f32, tag="scr")
                nc.vector.scalar_tensor_tensor(
                    scr[0:rows, :], wt[0:rows, :], 1.0, yc[0:rows, :], MULT, MULT,
                    accum_out=accB[0:rows, c : c + 1],
                )
            rt1 = pp.tile([1, 1], f32)
            nc.vector.reciprocal(rt1[:], pt[:])
            pb = psp.tile([128, 1], f32, tag="ps", name="pb")
            nc.tensor.matmul(pb[:], ones_row[:], rt1[:], start=True, stop=True)
            rtot = pp.tile([128, 1], f32)
            nc.vector.tensor_copy(rtot[:], pb[:])
            a32 = pp.tile([32, 128], f32)
            nc.vector.tensor_scalar_mul(a32[:], e32[:], rtot[0:32, :])
            nc.scalar.dma_start(attn_out.ap().rearrange("(q k) -> q k", q=32), a32[:])
            logits_sb = pp.tile([128, 32], f32)
            nc.vector.tensor_scalar_mul(accB[:], accB[:], rtot[:])
            nc.vector.tensor_add(logits_sb[:], accA[:], accB[:])
            nc.vector.tensor_add(logits_sb[:], logits_sb[:], ob_sb[:])
            nc.scalar.dma_start(
                logits_out.ap().rearrange("(p x) -> p x", p=128), logits_sb[:]
            )

    nc.compile()
    return nc


def _prep_inputs(word_input, last_hidden, encoder_hiddens, embedding,
                 w_ih, w_hh, b_ih, b_hh, attn_w, attn_b, out_w, out_b):
    word = int(np.asarray(word_input).reshape(-1)[0])
    x = np.asarray(embedding, np.float32)[word]
    h0 = np.asarray(last_hidden, np.float32).reshape(H)
    enc = np.ascontiguousarray(np.asarray(encoder_hiddens, np.float32).reshape(S, H))
    w_ih = np.asarray(w_ih, np.float32)
    w_hh = np.asarray(w_hh, np.float32)
    attn_w = np.asarray(attn_w, np.float32)
    out_w = np.asarray(out_w, np.float32)
    out_b = np.asarray(out_b, np.float32)
    b_ih = np.asarray(b_ih, np.float32)
    b_hh = np.asarray(b_hh, np.float32)

    xrep = np.ascontiguousarray(np.broadcast_to(x, (128, H)))
    h0rep = np.ascontiguousarray(np.broadcast_to(h0, (128, H)))

    in_maps = []
    for r in range(NCORES):
        rows = np.concatenate([
            np.arange(128 * r, 128 * (r + 1)),
            H + np.arange(128 * r, 128 * (r + 1)),
            2 * H + np.arange(128 * r, 128 * (r + 1)),
        ])
        E = enc[SSH * r : SSH * (r + 1)]
        obp = np.zeros(4096, np.float32)
        obp[:VSH] = out_b[VSH * r : VSH * (r + 1)]
        ow = out_w[VSH * r : VSH * (r + 1)]
        owcpad = np.zeros((1024, H), np.float32)
        owcpad[: VSH - 3072] = ow[3072:, H + PH]
        owcpe = np.ascontiguousarray(
            owcpad.reshape(8, 128, 128, 8).transpose(0, 3, 2, 1)
        )
        in_maps.append({
            "xrep_in": xrep,
            "h0rep_in": h0rep,
            "h0ch_in": np.ascontiguousarray(
                h0[128 * r : 128 * (r + 1)].reshape(128, 1)
            ),
            "wih_in": np.ascontiguousarray(w_ih[rows].reshape(3, 128, H)),
            "whh_in": np.ascontiguousarray(w_hh[rows].reshape(3, 128, H)),
            "bih_in": np.ascontiguousarray(b_ih[rows].reshape(3, 128).T),
            "bhh_in": np.ascontiguousarray(b_hh[rows].reshape(3, 128).T),
            "aw_in": np.ascontiguousarray(attn_w[128 * r : 128 * (r + 1), :]),
            "enc1_in": np.ascontiguousarray(E[:, PH]),
            "ect_in": np.ascontiguousarray(E[SPERM].T.reshape(8, 128, SSH)),
            "owh_in": np.ascontiguousarray(ow[:, 0:H]),
            "owc_in": np.ascontiguousarray(ow[:3072, H + PH]),
            "owcpe_in": owcpe,
            "ob_in": np.ascontiguousarray(obp.reshape(32, 128).T),
        })
    return in_maps


def _assemble(results):
    """results: list of per-core dicts {logits, h_out, attn_out} in device
    order -> full (logits[1,V], h[1,1,H], attn[1,1,S]) in natural order."""
    logits = np.empty(V, np.float32)
    for r in range(NCORES):
        d = np.asarray(results[r]["logits"], np.float32).reshape(128, 32)
        logits[VSH * r : VSH * (r + 1)] = d.T.ravel()[:VSH]
    h = np.asarray(results[0]["h_out"], np.float32)
    ad = np.asarray(results[0]["attn_out"], np.float32)
    attn = np.empty(S, np.float32)
    for r in range(NCORES):
        attn[SSH * r + SPERM] = ad[SSH * r : SSH * (r + 1)]
    return (
        logits.reshape(1, V),
        np.ascontiguousarray(h).reshape(1, 1, H),
        attn.reshape(1, 1, S),
    )


def kernel(**inputs):
    global LAST_RESULT
    if "nc" not in _CACHE:
        _CACHE["nc"] = _build()
    nc = _CACHE["nc"]
    in_maps = _prep_inputs(**inputs)
    trace = os.environ.get("ATTN_KERNEL_TRACE", "0") == "1"
    res = bass_utils.run_bass_kernel_spmd(
        nc, in_maps, core_ids=list(range(NCORES)), trace=trace
    )
    LAST_RESULT = res
    if trace and res.exec_time_ns is not None:
        print(f"HW exec time: {res.exec_time_ns} ns")
    return _assemble(res.results)
